# revision 6
# baseline (speedup 1.0000x reference)
# Trainium2 Bass kernel for nn_Invo2D (involution-style dynamic conv).
#
# Math (verified vs reference):
#   t1 = x @ W1 + b1                      [pix, 64]
#   t2 = t1 @ W2 + b2                     [pix, 144] = [g:16, j:9]
#   P[pix, f] = 3x3 SAME patches, f = tap*256 + ch   (tap row-major)
#   out[pix, co] = sum_j t2[pix, 9*(co//16)+j] * P[pix, 9*co+j]
#
# Sharding: data-parallel over batch, 1 image per NeuronCore (8 cores).
#
# Per-core layout: partition p = wq*64 + h (wq = w//32), per-partition free dim
# holds the 32 pixels (wl = w%32) of that image row-half.  Spatial taps become
# free-dim offsets (w) plus partition-shifted copies of x (h +- 1), so the
# data-dependent multiply-reduce runs lane-local on the Vector engine.
#
# v2: pipelined per-chunk (tiles for chunk c emitted just before chunk c's
# involution), F1 fold on the Pool engine, tap-8 gather on Act, planar bf16
# final + bf16 output DMA (host upcasts), persistent t1t bias rows.

import numpy as np
import ml_dtypes

H, W, C = 64, 64, 256
G, GC, KK = 16, 16, 9
M144, D = 144, 64
NCORES = 8
SLOTS = 34            # w slots per partition: slot s <-> w = 32*wq + s - 1
XF = SLOTS * C        # 8704 x-elems per partition
WLC = 8               # wl chunk size
NCHUNK = 32 // WLC    # 4 chunks
M16F = WLC * 4096     # product-chunk free size
SOFF = 4              # product slot offset: tap j -> slot j+4 (slots 4..12)

_cache = {}

F1_ON_POOL = True     # F1 fold on gpsimd (Pool) instead of DVE
TAP8_ON_ACT = True    # tap-8 strided gather on Act instead of DVE


def _rect_decomp(r0, r1):
    """[r0, r1) in (gc, j) space (gc = r//9, j = r%9) -> rects (gc0, ngc, j0, nj)."""
    out = []
    gc0, j0 = divmod(r0, 9)
    if j0 != 0:
        end = min(r1, (gc0 + 1) * 9)
        out.append((gc0, 1, j0, end - r0))
        r0 = end
        if r0 == r1:
            return out
        gc0, j0 = divmod(r0, 9)
    nfull = (r1 - r0) // 9
    if nfull:
        out.append((gc0, nfull, 0, 9))
        r0 += nfull * 9
        gc0 += nfull
    if r0 < r1:
        out.append((gc0, 1, 0, r1 - r0))
    return out


def _build_pieces():
    """Mult pieces: (g, gc0, ngc, j0, nj, tap). Each is one contiguous f-run
    within one spatial tap, rectangular in (gc, j)."""
    pieces = []
    for g in range(16):
        f_lo, f_hi = 144 * g, 144 * g + 144
        cuts = [f_lo] + [256 * k for k in range(1, 9) if f_lo < 256 * k < f_hi] + [f_hi]
        for a, b in zip(cuts, cuts[1:]):
            tap = a // 256
            for (gc0, ngc, j0, nj) in _rect_decomp(a - 144 * g, b - 144 * g):
                pieces.append((g, gc0, ngc, j0, nj, tap))
    return pieces


def _build_program():
    import concourse.bass as bass
    import concourse.tile as tile
    from concourse import bacc, mybir
    from concourse.masks import make_identity

    f32 = mybir.dt.float32
    bf16 = mybir.dt.bfloat16
    AP = bass.AP

    nc = bacc.Bacc(None, target_bir_lowering=False)
    x_d = nc.dram_tensor("x", [H, W, C], f32, kind="ExternalInput")
    w1_d = nc.dram_tensor("w1", [2, 128, D], bf16, kind="ExternalInput")
    b1_d = nc.dram_tensor("b1row", [1, D], bf16, kind="ExternalInput")
    w2_d = nc.dram_tensor("w2aug", [D + 1, M144], bf16, kind="ExternalInput")
    out_d = nc.dram_tensor("out", [H, W, C], bf16, kind="ExternalOutput")

    PIECES = _build_pieces()

    with tile.TileContext(nc) as tc:
        with (
            tc.tile_pool(name="singles", bufs=1) as singles,
            tc.tile_pool(name="xbufs", bufs=1) as xbufs,
            tc.tile_pool(name="big", bufs=1) as big,
            tc.tile_pool(name="folds", bufs=1) as folds,
            tc.tile_pool(name="outp", bufs=2) as outp,
            tc.tile_pool(name="pst", bufs=2, space="PSUM") as pst,
            tc.tile_pool(name="ps1", bufs=2, space="PSUM") as ps1p,
            tc.tile_pool(name="ps2", bufs=2, space="PSUM") as ps2p,
        ):
            # ---- constants ----
            ident = singles.tile([128, 128], bf16)
            make_identity(nc, ident[:])
            w1a = singles.tile([128, D], bf16)
            w1b = singles.tile([128, D], bf16)
            nc.sync.dma_start(out=w1a[:], in_=w1_d[0])
            nc.sync.dma_start(out=w1b[:], in_=w1_d[1])
            b1sb = singles.tile([1, D], bf16)
            nc.sync.dma_start(out=b1sb[:], in_=b1_d[:])
            w2sb = singles.tile([D + 1, M144], bf16)
            nc.sync.dma_start(out=w2sb[:], in_=w2_d[:])
            ones1 = singles.tile([1, 128], bf16)
            nc.gpsimd.memset(ones1[:], 1.0)
            z256 = singles.tile([1, 256], bf16)
            nc.gpsimd.memset(z256[:], 0.0)

            # persistent t1t buffers with pre-set bias row (1.0 at row D)
            t1ts = [singles.tile([D + 1, 128], bf16, name=f"t1t{i}")
                    for i in range(3)]
            for t1t in t1ts:
                nc.gpsimd.memset(t1t[D:D + 1, :], 1.0)

            # ---- x buffers (full-size; filled in per-chunk slot quarters) ----
            X0 = xbufs.tile([128, XF], bf16)
            XU = xbufs.tile([128, XF], bf16)   # row h+1
            XD = xbufs.tile([128, XF], bf16)   # row h-1
            # edge rows zero once (shift DMAs never touch these partitions)
            zsrc = AP(z256.tensor, 0, [[256, 1], [0, SLOTS], [1, 256]])
            nc.sync.dma_start(out=AP(XU.tensor, 63 * XF, [[XF, 1], [256, SLOTS], [1, 256]]),
                              in_=zsrc)
            nc.sync.dma_start(out=AP(XU.tensor, 127 * XF, [[XF, 1], [256, SLOTS], [1, 256]]),
                              in_=zsrc)
            nc.gpsimd.memset(AP(XD.tensor, 0, [[XF, 1], [1, XF]]), 0.0)
            nc.gpsimd.memset(AP(XD.tensor, 64 * XF, [[XF, 1], [1, XF]]), 0.0)

            xcm = singles.tile([128, 2 * 32 * 128], bf16)
            W16c = [big.tile([128, WLC * 256], bf16, name=f"w16_{i}",
                             tag=f"w16_{i}") for i in range(2)]
            M16 = big.tile([128, M16F], bf16, tag="m16")
            PB = folds.tile([128, WLC * 512], bf16)
            P3 = folds.tile([128, WLC * 256], bf16)
            M8s = [folds.tile([128, WLC * 256], bf16, name=f"m8_{i}")
                   for i in range(2)]

            # slot quarters: Q[c] = slots [lo, hi)
            QUART = [(0, 10), (10, 18), (18, 26), (26, 34)]
            XBUF = {-1: XD, 0: X0, 1: XU}

            for c in range(NCHUNK):
                lo, hi = QUART[c]
                n = hi - lo
                QF = n * 256
                # ---- stage DMA for this slot quarter (both wq halves) ----
                # wq0: slot s <-> w = s-1 (slots 1..33); wq1: slot s <-> w=31+s
                stq = outp.tile([128, 10 * 256], f32, name=f"stage{c}",
                                tag="stage")
                a0 = max(lo, 1)
                if a0 < hi:
                    nc.sync.dma_start(
                        out=AP(stq.tensor, (a0 - lo) * 256,
                               [[10 * 256, 64], [1, (hi - a0) * 256]]),
                        in_=AP(x_d, (a0 - 1) * 256,
                               [[W * C, 64], [1, (hi - a0) * 256]]),
                    )
                b1_ = min(hi, 33)
                if lo < b1_:
                    nc.sync.dma_start(
                        out=AP(stq.tensor, 64 * 10 * 256,
                               [[10 * 256, 64], [1, (b1_ - lo) * 256]]),
                        in_=AP(x_d, (31 + lo) * 256,
                               [[W * C, 64], [1, (b1_ - lo) * 256]]),
                    )
                # ---- cast quarter f32 -> bf16 on Act ----
                nc.scalar.copy(
                    out=AP(X0.tensor, lo * 256, [[XF, 128], [1, QF]]),
                    in_=AP(stq.tensor, 0, [[10 * 256, 128], [1, QF]]),
                )
                if c == 0:
                    # zero pads: wq0 slot0 (w=-1)
                    nc.gpsimd.memset(
                        AP(X0.tensor, 0, [[XF, 64], [1, 256]]), 0.0)
                if c == NCHUNK - 1:
                    # wq1 slot33 (w=64)
                    nc.gpsimd.memset(
                        AP(X0.tensor, 64 * XF + 33 * 256, [[XF, 64], [1, 256]]),
                        0.0)
                # ---- shifted copies for this quarter ----
                for half in range(2):
                    pbase = half * 64 * XF
                    nc.sync.dma_start(
                        out=AP(XU.tensor, pbase + lo * 256,
                               [[XF, 63], [1, n * 256]]),
                        in_=AP(X0.tensor, pbase + XF + lo * 256,
                               [[XF, 63], [1, n * 256]]))
                    nc.sync.dma_start(
                        out=AP(XD.tensor, pbase + XF + lo * 256,
                               [[XF, 63], [1, n * 256]]),
                        in_=AP(X0.tensor, pbase + lo * 256,
                               [[XF, 63], [1, n * 256]]))

                # ---- transposes for this chunk's tiles ----
                for half in range(2):
                    pt = pst.tile([128, 1024], bf16)
                    for tt in range(8):
                        t = c * 8 + tt
                        nc.tensor.transpose(
                            out=pt[:, tt * 128:(tt + 1) * 128],
                            in_=AP(X0.tensor, (t + 1) * 256 + half * 128,
                                   [[XF, 128], [1, 128]]),
                            identity=ident[:],
                        )
                    base = (half * 32 + c * 8) * 128
                    nc.scalar.copy(out=xcm[:, base:base + 8 * 128], in_=pt[:])

                # ---- this chunk's tiles: t1, t2, scatter into W16 ----
                w16 = W16c[c % 2]
                for tt in range(8):
                    t = c * 8 + tt
                    ps1 = ps1p.tile([D, 128], f32)
                    xc0 = AP(xcm.tensor, (0 * 32 + t) * 128,
                             [[2 * 32 * 128, 128], [1, 128]])
                    xc1 = AP(xcm.tensor, (1 * 32 + t) * 128,
                             [[2 * 32 * 128, 128], [1, 128]])
                    nc.tensor.matmul(ps1[:], lhsT=w1a[:], rhs=xc0,
                                     start=True, stop=False)
                    nc.tensor.matmul(ps1[:], lhsT=w1b[:], rhs=xc1,
                                     start=False, stop=False)
                    nc.tensor.matmul(ps1[:], lhsT=b1sb[:], rhs=ones1[:],
                                     start=False, stop=True)
                    t1t = t1ts[t % 3]
                    nc.scalar.copy(out=t1t[0:D, :], in_=ps1[:])
                    ps2 = ps2p.tile([128, M144], f32)
                    nc.tensor.matmul(ps2[:], lhsT=t1t[:], rhs=w2sb[:],
                                     start=True, stop=True)
                    # scatter t2[m=9g+j] into slots [wl, 16g + SOFF + j]
                    nc.scalar.copy(
                        out=AP(w16.tensor, tt * 256 + SOFF,
                               [[WLC * 256, 128], [16, 16], [1, 9]]),
                        in_=AP(ps2.tensor, 0, [[M144, 128], [9, 16], [1, 9]]),
                    )

                # ---- involution products (DVE) into M16 slots 4..12 ----
                for (g, gc0, ngc, j0, nj, tap) in PIECES:
                    di, dj = tap // 3 - 1, tap % 3 - 1
                    xb = XBUF[di]
                    ch0 = 144 * g + 9 * gc0 + j0 - 256 * tap
                    in0 = AP(xb.tensor, (c * WLC + dj + 1) * 256 + ch0,
                             [[XF, 128], [256, WLC], [9, ngc], [1, nj]])
                    in1 = AP(w16.tensor, 16 * g + SOFF + j0,
                             [[WLC * 256, 128], [256, WLC], [0, ngc], [1, nj]])
                    o = AP(M16.tensor, (16 * g + gc0) * 16 + SOFF + j0,
                           [[M16F, 128], [4096, WLC], [16, ngc], [1, nj]])
                    nc.vector.tensor_mul(o, in0, in1)

                # ---- folds ----
                # F1: slots (4..7) + (8..11) -> slots 0..3
                f1eng = nc.gpsimd if F1_ON_POOL else nc.vector
                f1eng.tensor_add(
                    AP(M16.tensor, 0, [[M16F, 128], [4096, WLC], [16, 256], [1, 4]]),
                    AP(M16.tensor, 4, [[M16F, 128], [4096, WLC], [16, 256], [1, 4]]),
                    AP(M16.tensor, 8, [[M16F, 128], [4096, WLC], [16, 256], [1, 4]]),
                )
                # F2: slots (0,1) + (2,3) -> pair buffer [wl*512 + 2*co + s]
                nc.vector.tensor_add(
                    AP(PB.tensor, 0, [[WLC * 512, 128], [512, WLC], [2, 256], [1, 2]]),
                    AP(M16.tensor, 0, [[M16F, 128], [4096, WLC], [16, 256], [1, 2]]),
                    AP(M16.tensor, 2, [[M16F, 128], [4096, WLC], [16, 256], [1, 2]]),
                )
                # F3: pair buffer -> planar P3
                nc.vector.tensor_add(
                    AP(P3.tensor, 0, [[WLC * 256, 128], [256, WLC], [1, 256]]),
                    AP(PB.tensor, 0, [[WLC * 512, 128], [512, WLC], [2, 256]]),
                    AP(PB.tensor, 1, [[WLC * 512, 128], [512, WLC], [2, 256]]),
                )
                # tap-8 gather: M16 slot 12 -> planar M8
                M8 = M8s[c % 2]
                if TAP8_ON_ACT:
                    nc.scalar.copy(
                        out=AP(M8.tensor, 0, [[WLC * 256, 128], [256, WLC], [1, 256]]),
                        in_=AP(M16.tensor, 12, [[M16F, 128], [4096, WLC], [16, 256]]),
                    )
                else:
                    nc.vector.tensor_copy(
                        out=AP(M8.tensor, 0, [[WLC * 256, 128], [256, WLC], [1, 256]]),
                        in_=AP(M16.tensor, 12, [[M16F, 128], [4096, WLC], [16, 256]]),
                    )
                # final: planar + planar -> bf16 out staging
                outc = outp.tile([128, WLC * 256], bf16)
                nc.vector.tensor_add(
                    AP(outc.tensor, 0, [[WLC * 256, 128], [1, WLC * 256]]),
                    AP(P3.tensor, 0, [[WLC * 256, 128], [1, WLC * 256]]),
                    AP(M8.tensor, 0, [[WLC * 256, 128], [1, WLC * 256]]),
                )
                wl0 = c * WLC
                nc.sync.dma_start(
                    out=AP(out_d, wl0 * 256, [[W * C, 64], [1, WLC * 256]]),
                    in_=AP(outc.tensor, 0, [[WLC * 256, 64], [1, WLC * 256]]),
                )
                nc.sync.dma_start(
                    out=AP(out_d, (32 + wl0) * 256, [[W * C, 64], [1, WLC * 256]]),
                    in_=AP(outc.tensor, 64 * WLC * 256,
                           [[WLC * 256, 64], [1, WLC * 256]]),
                )
    nc.compile()
    return nc


def _get_program():
    if "nc" not in _cache:
        _cache["nc"] = _build_program()
    return _cache["nc"]


def kernel(x, W1, b1, W2, b2, trace=False):
    from concourse.bass_utils import run_bass_kernel_spmd

    nc = _get_program()
    bf = ml_dtypes.bfloat16
    w1_h = np.ascontiguousarray(W1.astype(bf).reshape(2, 128, D))
    b1_h = np.ascontiguousarray(b1.astype(bf).reshape(1, D))
    w2_h = np.ascontiguousarray(
        np.concatenate([W2, b2[None, :]], axis=0).astype(bf))
    in_maps = [
        {
            "x": np.ascontiguousarray(x[i], dtype=np.float32),
            "w1": w1_h,
            "b1row": b1_h,
            "w2aug": w2_h,
        }
        for i in range(NCORES)
    ]
    res = run_bass_kernel_spmd(nc, in_maps, core_ids=list(range(NCORES)),
                               trace=trace)
    out = np.stack([res.results[i]["out"] for i in range(NCORES)], axis=0)
    out = out.astype(np.float32)
    if trace:
        return out, res
    return out


# revision 8
# speedup vs baseline: 1.2707x; 1.2707x over previous
# Trainium2 Bass kernel for nn_Invo2D (involution-style dynamic conv).
#
# Math (verified vs reference):
#   t1 = x @ W1 + b1                      [pix, 64]
#   t2 = t1 @ W2 + b2                     [pix, 144] = [g:16, j:9]
#   P[pix, f] = 3x3 SAME patches, f = tap*256 + ch   (tap row-major)
#   out[pix, co] = sum_j t2[pix, 9*(co//16)+j] * P[pix, 9*co+j]
#
# Sharding: data-parallel over batch, 1 image per NeuronCore (8 cores).
#
# Per-core layout: partition p = wq*64 + h (wq = w//32), per-partition free dim
# holds the 32 pixels (wl = w%32) of that image row-half.  Spatial taps become
# free-dim offsets (w) plus row-shifted copies of x (h +- 1), so the
# data-dependent multiply-reduce runs lane-local on the Vector engine.
#
# v3: host pre-arranges x (bf16 slot images x0/xu/xd + channel-major xcm), so
# the device program is just: per chunk, DMA quarters in, 4 matmuls/tile for
# t2, elementwise products (split DVE/Pool), fold tree, bf16 out DMA.  Host
# upcasts the bf16 output to f32.

import numpy as np
import ml_dtypes

H, W, C = 64, 64, 256
G, GC, KK = 16, 16, 9
M144, D = 144, 64
NCORES = 8
SLOTS = 34            # w slots per partition: slot s <-> w = 32*wq + s - 1
XF = SLOTS * C        # 8704 x-elems per partition
WLC = 8               # wl chunk size
NCHUNK = 32 // WLC    # 4 chunks
M16F = WLC * 4096     # product-chunk free size
SOFF = 4              # product slot offset: tap j -> slot j+4 (slots 4..12)
POOL_ELEMS = 7600     # per-chunk product elems (free) assigned to Pool engine

_cache = {}


def _rect_decomp(r0, r1):
    """[r0, r1) in (gc, j) space (gc = r//9, j = r%9) -> rects (gc0, ngc, j0, nj)."""
    out = []
    gc0, j0 = divmod(r0, 9)
    if j0 != 0:
        end = min(r1, (gc0 + 1) * 9)
        out.append((gc0, 1, j0, end - r0))
        r0 = end
        if r0 == r1:
            return out
        gc0, j0 = divmod(r0, 9)
    nfull = (r1 - r0) // 9
    if nfull:
        out.append((gc0, nfull, 0, 9))
        r0 += nfull * 9
        gc0 += nfull
    if r0 < r1:
        out.append((gc0, 1, 0, r1 - r0))
    return out


def _build_pieces():
    """Mult pieces: (g, gc0, ngc, j0, nj, tap). Each is one contiguous f-run
    within one spatial tap, rectangular in (gc, j)."""
    pieces = []
    for g in range(16):
        f_lo, f_hi = 144 * g, 144 * g + 144
        cuts = [f_lo] + [256 * k for k in range(1, 9) if f_lo < 256 * k < f_hi] + [f_hi]
        for a, b in zip(cuts, cuts[1:]):
            tap = a // 256
            for (gc0, ngc, j0, nj) in _rect_decomp(a - 144 * g, b - 144 * g):
                pieces.append((g, gc0, ngc, j0, nj, tap))
    return pieces


def _split_pieces(pieces):
    """Assign the largest pieces to the Pool engine up to POOL_ELEMS/wl-chunk."""
    order = sorted(range(len(pieces)),
                   key=lambda i: -(pieces[i][2] * pieces[i][4]))
    pool_idx = set()
    budget = POOL_ELEMS // WLC
    for i in order:
        sz = pieces[i][2] * pieces[i][4]
        if sz <= budget:
            pool_idx.add(i)
            budget -= sz
        if budget <= 0:
            break
    dve = [p for i, p in enumerate(pieces) if i not in pool_idx]
    pool = [p for i, p in enumerate(pieces) if i in pool_idx]
    return dve, pool


def _build_program():
    import concourse.bass as bass
    import concourse.tile as tile
    from concourse import bacc, mybir

    f32 = mybir.dt.float32
    bf16 = mybir.dt.bfloat16
    AP = bass.AP

    nc = bacc.Bacc(None, target_bir_lowering=False)
    x0_d = nc.dram_tensor("x0", [128, XF], bf16, kind="ExternalInput")
    xu_d = nc.dram_tensor("xu", [128, XF], bf16, kind="ExternalInput")
    xd_d = nc.dram_tensor("xd", [128, XF], bf16, kind="ExternalInput")
    xcm_d = nc.dram_tensor("xcm", [128, 8192], bf16, kind="ExternalInput")
    w1_d = nc.dram_tensor("w1", [2, 128, D], bf16, kind="ExternalInput")
    b1_d = nc.dram_tensor("b1row", [1, D], bf16, kind="ExternalInput")
    w2_d = nc.dram_tensor("w2aug", [D + 1, M144], bf16, kind="ExternalInput")
    out_d = nc.dram_tensor("out", [H, W, C], bf16, kind="ExternalOutput")

    DVE_PIECES, POOL_PIECES = _split_pieces(_build_pieces())

    with tile.TileContext(nc) as tc:
        with (
            tc.tile_pool(name="singles", bufs=1) as singles,
            tc.tile_pool(name="xbufs", bufs=1) as xbufs,
            tc.tile_pool(name="big", bufs=1) as big,
            tc.tile_pool(name="folds", bufs=1) as folds,
            tc.tile_pool(name="outp", bufs=2) as outp,
            tc.tile_pool(name="ps1", bufs=2, space="PSUM") as ps1p,
            tc.tile_pool(name="ps2", bufs=2, space="PSUM") as ps2p,
        ):
            # ---- constants ----
            w1a = singles.tile([128, D], bf16)
            w1b = singles.tile([128, D], bf16)
            nc.sync.dma_start(out=w1a[:], in_=w1_d[0])
            nc.sync.dma_start(out=w1b[:], in_=w1_d[1])
            b1sb = singles.tile([1, D], bf16)
            nc.sync.dma_start(out=b1sb[:], in_=b1_d[:])
            w2sb = singles.tile([D + 1, M144], bf16)
            nc.sync.dma_start(out=w2sb[:], in_=w2_d[:])
            ones1 = singles.tile([1, 128], bf16)
            nc.gpsimd.memset(ones1[:], 1.0)

            # persistent t1t buffers with pre-set bias row (1.0 at row D)
            t1ts = [singles.tile([D + 1, 128], bf16, name=f"t1t{i}")
                    for i in range(3)]
            for t1t in t1ts:
                nc.gpsimd.memset(t1t[D:D + 1, :], 1.0)

            X0 = xbufs.tile([128, XF], bf16)
            XU = xbufs.tile([128, XF], bf16)   # row h+1
            XD = xbufs.tile([128, XF], bf16)   # row h-1
            xcm = singles.tile([128, 8192], bf16)
            W16c = [big.tile([128, WLC * 256], bf16, name=f"w16_{i}",
                             tag=f"w16_{i}") for i in range(2)]
            M16 = big.tile([128, M16F], bf16, tag="m16")
            F1b = folds.tile([128, WLC * 1024], bf16)
            F2b = folds.tile([128, WLC * 512], bf16)
            P3 = folds.tile([128, WLC * 256], bf16)
            M8s = [folds.tile([128, WLC * 256], bf16, name=f"m8_{i}")
                   for i in range(2)]

            # slot quarters: Q[c] = slots [lo, hi)
            QUART = [(0, 10), (10, 18), (18, 26), (26, 34)]
            XBUF = {-1: XD, 0: X0, 1: XU}

            for c in range(NCHUNK):
                lo, hi = QUART[c]
                QF = (hi - lo) * 256
                # ---- x quarters in: x0 + xcm first (tiles), then xu/xd ----
                nc.sync.dma_start(
                    out=AP(X0.tensor, lo * 256, [[XF, 128], [1, QF]]),
                    in_=AP(x0_d, lo * 256, [[XF, 128], [1, QF]]))
                nc.scalar.dma_start(
                    out=AP(xcm.tensor, c * 8 * 128,
                           [[8192, 128], [4096, 2], [1, 1024]]),
                    in_=AP(xcm_d, c * 8 * 128,
                           [[8192, 128], [4096, 2], [1, 1024]]))

                # ---- this chunk's tiles: t1, t2, scatter into W16 ----
                w16 = W16c[c % 2]
                for tt in range(8):
                    t = c * 8 + tt
                    ps1 = ps1p.tile([D, 128], f32)
                    xc0 = AP(xcm.tensor, (0 * 32 + t) * 128,
                             [[8192, 128], [1, 128]])
                    xc1 = AP(xcm.tensor, (1 * 32 + t) * 128,
                             [[8192, 128], [1, 128]])
                    nc.tensor.matmul(ps1[:], lhsT=w1a[:], rhs=xc0,
                                     start=True, stop=False)
                    nc.tensor.matmul(ps1[:], lhsT=w1b[:], rhs=xc1,
                                     start=False, stop=False)
                    nc.tensor.matmul(ps1[:], lhsT=b1sb[:], rhs=ones1[:],
                                     start=False, stop=True)
                    t1t = t1ts[t % 3]
                    nc.scalar.copy(out=t1t[0:D, :], in_=ps1[:])
                    ps2 = ps2p.tile([128, M144], f32)
                    nc.tensor.matmul(ps2[:], lhsT=t1t[:], rhs=w2sb[:],
                                     start=True, stop=True)
                    # scatter t2[m=9g+j] into slots [wl, 16g + SOFF + j]
                    nc.scalar.copy(
                        out=AP(w16.tensor, tt * 256 + SOFF,
                               [[WLC * 256, 128], [16, 16], [1, 9]]),
                        in_=AP(ps2.tensor, 0, [[M144, 128], [9, 16], [1, 9]]),
                    )

                # xu/xd quarters (needed by products only)
                nc.sync.dma_start(
                    out=AP(XU.tensor, lo * 256, [[XF, 128], [1, QF]]),
                    in_=AP(xu_d, lo * 256, [[XF, 128], [1, QF]]))
                nc.sync.dma_start(
                    out=AP(XD.tensor, lo * 256, [[XF, 128], [1, QF]]),
                    in_=AP(xd_d, lo * 256, [[XF, 128], [1, QF]]))

                # ---- involution products into M16 slots 4..12 ----
                def emit_piece(eng, g, gc0, ngc, j0, nj, tap):
                    di, dj = tap // 3 - 1, tap % 3 - 1
                    xb = XBUF[di]
                    ch0 = 144 * g + 9 * gc0 + j0 - 256 * tap
                    in0 = AP(xb.tensor, (c * WLC + dj + 1) * 256 + ch0,
                             [[XF, 128], [256, WLC], [9, ngc], [1, nj]])
                    in1 = AP(w16.tensor, 16 * g + SOFF + j0,
                             [[WLC * 256, 128], [256, WLC], [0, ngc], [1, nj]])
                    o = AP(M16.tensor, (16 * g + gc0) * 16 + SOFF + j0,
                           [[M16F, 128], [4096, WLC], [16, ngc], [1, nj]])
                    eng.tensor_mul(o, in0, in1)

                for pc in POOL_PIECES:
                    emit_piece(nc.gpsimd, *pc)
                for pc in DVE_PIECES:
                    emit_piece(nc.vector, *pc)

                # ---- folds (compact buffers, as in baseline) ----
                # F1: slots (4..7) + (8..11) -> F1b[wl*1024 + 4*co + s]
                nc.vector.tensor_add(
                    AP(F1b.tensor, 0, [[WLC * 1024, 128], [1024, WLC], [4, 256], [1, 4]]),
                    AP(M16.tensor, SOFF, [[M16F, 128], [4096, WLC], [16, 256], [1, 4]]),
                    AP(M16.tensor, SOFF + 4, [[M16F, 128], [4096, WLC], [16, 256], [1, 4]]),
                )
                # F2: F1b pairs -> F2b[wl*512 + 2*co + s]
                nc.vector.tensor_add(
                    AP(F2b.tensor, 0, [[WLC * 512, 128], [512, WLC], [2, 256], [1, 2]]),
                    AP(F1b.tensor, 0, [[WLC * 1024, 128], [1024, WLC], [4, 256], [1, 2]]),
                    AP(F1b.tensor, 2, [[WLC * 1024, 128], [1024, WLC], [4, 256], [1, 2]]),
                )
                # F3: F2b pairs -> planar P3
                nc.vector.tensor_add(
                    AP(P3.tensor, 0, [[WLC * 256, 128], [256, WLC], [1, 256]]),
                    AP(F2b.tensor, 0, [[WLC * 512, 128], [512, WLC], [2, 256]]),
                    AP(F2b.tensor, 1, [[WLC * 512, 128], [512, WLC], [2, 256]]),
                )
                # tap-8 gather on Act: M16 slot 12 -> planar M8
                M8 = M8s[c % 2]
                nc.scalar.copy(
                    out=AP(M8.tensor, 0, [[WLC * 256, 128], [256, WLC], [1, 256]]),
                    in_=AP(M16.tensor, SOFF + 8,
                           [[M16F, 128], [4096, WLC], [16, 256]]),
                )
                # final: planar + planar -> bf16 out staging
                outc = outp.tile([128, WLC * 256], bf16)
                nc.vector.tensor_add(
                    AP(outc.tensor, 0, [[WLC * 256, 128], [1, WLC * 256]]),
                    AP(P3.tensor, 0, [[WLC * 256, 128], [1, WLC * 256]]),
                    AP(M8.tensor, 0, [[WLC * 256, 128], [1, WLC * 256]]),
                )
                wl0 = c * WLC
                nc.sync.dma_start(
                    out=AP(out_d, wl0 * 256, [[W * C, 64], [1, WLC * 256]]),
                    in_=AP(outc.tensor, 0, [[WLC * 256, 64], [1, WLC * 256]]),
                )
                nc.sync.dma_start(
                    out=AP(out_d, (32 + wl0) * 256, [[W * C, 64], [1, WLC * 256]]),
                    in_=AP(outc.tensor, 64 * WLC * 256,
                           [[WLC * 256, 64], [1, WLC * 256]]),
                )
    nc.compile()
    return nc


def _get_program():
    if "nc" not in _cache:
        _cache["nc"] = _build_program()
    return _cache["nc"]


def _host_x(x_img):
    """x_img [H, W, C] f32 -> (x0, xu, xd, xcm) bf16 device images."""
    bf = ml_dtypes.bfloat16
    xb = x_img.astype(bf)

    def slot_img(src):
        img = np.zeros((128, SLOTS, C), dtype=bf)
        img[0:64, 1:34, :] = src[:, 0:33, :]      # wq0: slot s <-> w = s-1
        img[64:128, 0:33, :] = src[:, 31:64, :]   # wq1: slot s <-> w = 31+s
        return img.reshape(128, XF)

    zr = np.zeros((1, W, C), dtype=bf)
    x0 = slot_img(xb)
    xu = slot_img(np.concatenate([xb[1:], zr], axis=0))    # row h+1
    xd = slot_img(np.concatenate([zr, xb[:-1]], axis=0))   # row h-1
    # xcm[ch, (half*32+t)*128 + wq*64 + h] = x[h, 32wq+t, 128half+ch]
    a = xb.transpose(2, 1, 0)                  # [c, w, h]
    a = a.reshape(2, 128, 2, 32, H)            # [half, ch, wq, t, h]
    a = a.transpose(1, 0, 3, 2, 4)             # [ch, half, t, wq, h]
    xcm = np.ascontiguousarray(a.reshape(128, 8192))
    return x0, xu, xd, xcm


def kernel(x, W1, b1, W2, b2, trace=False):
    from concourse.bass_utils import run_bass_kernel_spmd

    nc = _get_program()
    bf = ml_dtypes.bfloat16
    w1_h = np.ascontiguousarray(W1.astype(bf).reshape(2, 128, D))
    b1_h = np.ascontiguousarray(b1.astype(bf).reshape(1, D))
    w2_h = np.ascontiguousarray(
        np.concatenate([W2, b2[None, :]], axis=0).astype(bf))
    in_maps = []
    for i in range(NCORES):
        x0, xu, xd, xcm = _host_x(np.asarray(x[i], dtype=np.float32))
        in_maps.append({
            "x0": x0, "xu": xu, "xd": xd, "xcm": xcm,
            "w1": w1_h, "b1row": b1_h, "w2aug": w2_h,
        })
    res = run_bass_kernel_spmd(nc, in_maps, core_ids=list(range(NCORES)),
                               trace=trace)
    out = np.stack([res.results[i]["out"] for i in range(NCORES)], axis=0)
    out = out.astype(np.float32)
    if trace:
        return out, res
    return out


# revision 10
# speedup vs baseline: 1.4797x; 1.1644x over previous
# Trainium2 Bass kernel for nn_Invo2D (involution-style dynamic conv).
#
# Math (verified vs reference):
#   t1 = x @ W1 + b1                      [pix, 64]
#   t2 = t1 @ W2 + b2                     [pix, 144] = [g:16, j:9]
#   P[pix, f] = 3x3 SAME patches, f = tap*256 + ch   (tap row-major)
#   out[pix, co] = sum_j t2[pix, 9*(co//16)+j] * P[pix, 9*co+j]
#
# Sharding: data-parallel over batch, 1 image per NeuronCore (8 cores).
#
# Per-core layout: partition p = wq*64 + h (wq = w//32), per-partition free dim
# holds the 32 pixels (wl = w%32) of that image row-half.  Spatial taps become
# free-dim offsets (w) plus row-shifted copies of x (h +- 1), so the
# data-dependent multiply-reduce runs lane-local on the Vector engine.
#
# v3: host pre-arranges x (bf16 slot images x0/xu/xd + channel-major xcm), so
# the device program is just: per chunk, DMA quarters in, 4 matmuls/tile for
# t2, elementwise products (split DVE/Pool), fold tree, bf16 out DMA.  Host
# upcasts the bf16 output to f32.

import numpy as np
import ml_dtypes

H, W, C = 64, 64, 256
G, GC, KK = 16, 16, 9
M144, D = 144, 64
NCORES = 8
SLOTS = 34            # w slots per partition: slot s <-> w = 32*wq + s - 1
XF = SLOTS * C        # 8704 x-elems per partition
WLC = 8               # wl chunk size
NCHUNK = 32 // WLC    # 4 chunks
M16F = WLC * 4096     # product-chunk free size
SOFF = 4              # product slot offset: tap j -> slot j+4 (slots 4..12)
import os as _os
POOL_ELEMS = int(_os.environ.get("POOL_ELEMS", "4608"))  # per-chunk free elems on Pool

_cache = {}


def _rect_decomp(r0, r1):
    """[r0, r1) in (gc, j) space (gc = r//9, j = r%9) -> rects (gc0, ngc, j0, nj)."""
    out = []
    gc0, j0 = divmod(r0, 9)
    if j0 != 0:
        end = min(r1, (gc0 + 1) * 9)
        out.append((gc0, 1, j0, end - r0))
        r0 = end
        if r0 == r1:
            return out
        gc0, j0 = divmod(r0, 9)
    nfull = (r1 - r0) // 9
    if nfull:
        out.append((gc0, nfull, 0, 9))
        r0 += nfull * 9
        gc0 += nfull
    if r0 < r1:
        out.append((gc0, 1, 0, r1 - r0))
    return out


def _build_pieces():
    """Mult pieces: (g, gc0, ngc, j0, nj, tap).

    Within one ki row (3 horizontal taps), x addressing is linear in f: the
    w-taps are adjacent 256-channel columns, so one op can span 256-boundaries
    (the AP walks into the next w column, which is exactly the next tap's
    data).  Pieces therefore split only at ki-row changes (f = 768k), giving
    one full 16x9 rect per group except g=5 and g=10."""
    pieces = []
    for g in range(16):
        f0, f1 = 144 * g, 144 * g + 144
        kb = None
        for k in (768, 1536):
            if f0 < k < f1:
                kb = k
        if kb is None:
            pieces.append((g, 0, 16, 0, 9, f0 // 256))
        else:
            for (a, b) in ((f0, kb), (kb, f1)):
                for (gc0, ngc, j0, nj) in _rect_decomp(a - f0, b - f0):
                    tap = (f0 + 9 * gc0 + j0) // 256
                    pieces.append((g, gc0, ngc, j0, nj, tap))
    return pieces


def _split_pieces(pieces):
    """Assign the largest pieces to the Pool engine up to POOL_ELEMS/wl-chunk."""
    order = sorted(range(len(pieces)),
                   key=lambda i: -(pieces[i][2] * pieces[i][4]))
    pool_idx = set()
    budget = POOL_ELEMS // WLC
    for i in order:
        sz = pieces[i][2] * pieces[i][4]
        if sz <= budget:
            pool_idx.add(i)
            budget -= sz
        if budget <= 0:
            break
    dve = [p for i, p in enumerate(pieces) if i not in pool_idx]
    pool = [p for i, p in enumerate(pieces) if i in pool_idx]
    return dve, pool


def _build_program():
    import concourse.bass as bass
    import concourse.tile as tile
    from concourse import bacc, mybir

    f32 = mybir.dt.float32
    bf16 = mybir.dt.bfloat16
    AP = bass.AP

    nc = bacc.Bacc(None, target_bir_lowering=False)
    x0_d = nc.dram_tensor("x0", [128, XF], bf16, kind="ExternalInput")
    xu_d = nc.dram_tensor("xu", [128, XF], bf16, kind="ExternalInput")
    xd_d = nc.dram_tensor("xd", [128, XF], bf16, kind="ExternalInput")
    xcm_d = nc.dram_tensor("xcm", [128, 8192], bf16, kind="ExternalInput")
    w1_d = nc.dram_tensor("w1", [2, 128, D], bf16, kind="ExternalInput")
    b1_d = nc.dram_tensor("b1row", [1, D], bf16, kind="ExternalInput")
    w2_d = nc.dram_tensor("w2aug", [D + 1, M144], bf16, kind="ExternalInput")
    out_d = nc.dram_tensor("out", [H, W, C], bf16, kind="ExternalOutput")

    DVE_PIECES, POOL_PIECES = _split_pieces(_build_pieces())

    with tile.TileContext(nc) as tc:
        with (
            tc.tile_pool(name="singles", bufs=1) as singles,
            tc.tile_pool(name="xbufs", bufs=1) as xbufs,
            tc.tile_pool(name="big", bufs=1) as big,
            tc.tile_pool(name="folds", bufs=1) as folds,
            tc.tile_pool(name="outp", bufs=2) as outp,
            tc.tile_pool(name="ps1", bufs=2, space="PSUM") as ps1p,
            tc.tile_pool(name="ps2", bufs=2, space="PSUM") as ps2p,
        ):
            # ---- constants ----
            w1a = singles.tile([128, D], bf16)
            w1b = singles.tile([128, D], bf16)
            nc.sync.dma_start(out=w1a[:], in_=w1_d[0])
            nc.sync.dma_start(out=w1b[:], in_=w1_d[1])
            b1sb = singles.tile([1, D], bf16)
            nc.sync.dma_start(out=b1sb[:], in_=b1_d[:])
            w2sb = singles.tile([D + 1, M144], bf16)
            nc.sync.dma_start(out=w2sb[:], in_=w2_d[:])
            ones1 = singles.tile([1, 128], bf16)
            nc.gpsimd.memset(ones1[:], 1.0)

            # persistent t1t buffers with pre-set bias row (1.0 at row D)
            t1ts = [singles.tile([D + 1, 128], bf16, name=f"t1t{i}")
                    for i in range(3)]
            for t1t in t1ts:
                nc.gpsimd.memset(t1t[D:D + 1, :], 1.0)

            X0 = xbufs.tile([128, XF], bf16)
            XU = xbufs.tile([128, XF], bf16)   # row h+1
            XD = xbufs.tile([128, XF], bf16)   # row h-1
            xcm = singles.tile([128, 8192], bf16)
            W16c = [big.tile([128, WLC * 256], bf16, name=f"w16_{i}",
                             tag=f"w16_{i}") for i in range(2)]
            M16 = big.tile([128, M16F], bf16, tag="m16")
            F1b = folds.tile([128, WLC * 1024], bf16)
            F2b = folds.tile([128, WLC * 512], bf16)
            P3 = folds.tile([128, WLC * 256], bf16)
            M8s = [folds.tile([128, WLC * 256], bf16, name=f"m8_{i}")
                   for i in range(2)]

            # slot quarters: Q[c] = slots [lo, hi)
            QUART = [(0, 10), (10, 18), (18, 26), (26, 34)]
            XBUF = {-1: XD, 0: X0, 1: XU}

            for c in range(NCHUNK):
                lo, hi = QUART[c]
                QF = (hi - lo) * 256
                # ---- x quarters in: x0 + xcm first (tiles), then xu/xd ----
                nc.sync.dma_start(
                    out=AP(X0.tensor, lo * 256, [[XF, 128], [1, QF]]),
                    in_=AP(x0_d, lo * 256, [[XF, 128], [1, QF]]))
                nc.scalar.dma_start(
                    out=AP(xcm.tensor, c * 8 * 128,
                           [[8192, 128], [4096, 2], [1, 1024]]),
                    in_=AP(xcm_d, c * 8 * 128,
                           [[8192, 128], [4096, 2], [1, 1024]]))

                # ---- this chunk's tiles: t1, t2, scatter into W16 ----
                w16 = W16c[c % 2]
                for tt in range(8):
                    t = c * 8 + tt
                    ps1 = ps1p.tile([D, 128], f32)
                    xc0 = AP(xcm.tensor, (0 * 32 + t) * 128,
                             [[8192, 128], [1, 128]])
                    xc1 = AP(xcm.tensor, (1 * 32 + t) * 128,
                             [[8192, 128], [1, 128]])
                    nc.tensor.matmul(ps1[:], lhsT=w1a[:], rhs=xc0,
                                     start=True, stop=False)
                    nc.tensor.matmul(ps1[:], lhsT=w1b[:], rhs=xc1,
                                     start=False, stop=False)
                    nc.tensor.matmul(ps1[:], lhsT=b1sb[:], rhs=ones1[:],
                                     start=False, stop=True)
                    t1t = t1ts[t % 3]
                    nc.scalar.copy(out=t1t[0:D, :], in_=ps1[:])
                    ps2 = ps2p.tile([128, M144], f32)
                    nc.tensor.matmul(ps2[:], lhsT=t1t[:], rhs=w2sb[:],
                                     start=True, stop=True)
                    # scatter t2[m=9g+j] into slots [wl, 16g + SOFF + j]
                    nc.scalar.copy(
                        out=AP(w16.tensor, tt * 256 + SOFF,
                               [[WLC * 256, 128], [16, 16], [1, 9]]),
                        in_=AP(ps2.tensor, 0, [[M144, 128], [9, 16], [1, 9]]),
                    )

                # xu/xd quarters (needed by products only)
                nc.sync.dma_start(
                    out=AP(XU.tensor, lo * 256, [[XF, 128], [1, QF]]),
                    in_=AP(xu_d, lo * 256, [[XF, 128], [1, QF]]))
                nc.sync.dma_start(
                    out=AP(XD.tensor, lo * 256, [[XF, 128], [1, QF]]),
                    in_=AP(xd_d, lo * 256, [[XF, 128], [1, QF]]))

                # ---- involution products into M16 slots 4..12 ----
                def emit_piece(eng, g, gc0, ngc, j0, nj, tap):
                    di, dj = tap // 3 - 1, tap % 3 - 1
                    xb = XBUF[di]
                    ch0 = 144 * g + 9 * gc0 + j0 - 256 * tap
                    in0 = AP(xb.tensor, (c * WLC + dj + 1) * 256 + ch0,
                             [[XF, 128], [256, WLC], [9, ngc], [1, nj]])
                    in1 = AP(w16.tensor, 16 * g + SOFF + j0,
                             [[WLC * 256, 128], [256, WLC], [0, ngc], [1, nj]])
                    o = AP(M16.tensor, (16 * g + gc0) * 16 + SOFF + j0,
                           [[M16F, 128], [4096, WLC], [16, ngc], [1, nj]])
                    eng.tensor_mul(o, in0, in1)

                for pc in POOL_PIECES:
                    emit_piece(nc.gpsimd, *pc)
                for pc in DVE_PIECES:
                    emit_piece(nc.vector, *pc)

                # tap-8 gather (DVE): M16 slot 12 -> planar M8
                M8 = M8s[c % 2]
                nc.vector.tensor_copy(
                    out=AP(M8.tensor, 0, [[WLC * 256, 128], [256, WLC], [1, 256]]),
                    in_=AP(M16.tensor, SOFF + 8,
                           [[M16F, 128], [4096, WLC], [16, 256]]),
                )
                # ---- folds (compact buffers, as in baseline) ----
                # F1: slots (4..7) + (8..11) -> F1b[wl*1024 + 4*co + s]
                nc.vector.tensor_add(
                    AP(F1b.tensor, 0, [[WLC * 1024, 128], [1024, WLC], [4, 256], [1, 4]]),
                    AP(M16.tensor, SOFF, [[M16F, 128], [4096, WLC], [16, 256], [1, 4]]),
                    AP(M16.tensor, SOFF + 4, [[M16F, 128], [4096, WLC], [16, 256], [1, 4]]),
                )
                # F2: F1b pairs -> F2b[wl*512 + 2*co + s]
                nc.vector.tensor_add(
                    AP(F2b.tensor, 0, [[WLC * 512, 128], [512, WLC], [2, 256], [1, 2]]),
                    AP(F1b.tensor, 0, [[WLC * 1024, 128], [1024, WLC], [4, 256], [1, 2]]),
                    AP(F1b.tensor, 2, [[WLC * 1024, 128], [1024, WLC], [4, 256], [1, 2]]),
                )
                # F3: F2b pairs -> planar P3
                nc.vector.tensor_add(
                    AP(P3.tensor, 0, [[WLC * 256, 128], [256, WLC], [1, 256]]),
                    AP(F2b.tensor, 0, [[WLC * 512, 128], [512, WLC], [2, 256]]),
                    AP(F2b.tensor, 1, [[WLC * 512, 128], [512, WLC], [2, 256]]),
                )

                # final: planar + planar -> bf16 out staging
                outc = outp.tile([128, WLC * 256], bf16)
                nc.vector.tensor_add(
                    AP(outc.tensor, 0, [[WLC * 256, 128], [1, WLC * 256]]),
                    AP(P3.tensor, 0, [[WLC * 256, 128], [1, WLC * 256]]),
                    AP(M8.tensor, 0, [[WLC * 256, 128], [1, WLC * 256]]),
                )
                wl0 = c * WLC
                nc.sync.dma_start(
                    out=AP(out_d, wl0 * 256, [[W * C, 64], [1, WLC * 256]]),
                    in_=AP(outc.tensor, 0, [[WLC * 256, 64], [1, WLC * 256]]),
                )
                nc.sync.dma_start(
                    out=AP(out_d, (32 + wl0) * 256, [[W * C, 64], [1, WLC * 256]]),
                    in_=AP(outc.tensor, 64 * WLC * 256,
                           [[WLC * 256, 64], [1, WLC * 256]]),
                )
    nc.compile()
    return nc


def _get_program():
    if "nc" not in _cache:
        _cache["nc"] = _build_program()
    return _cache["nc"]


def _host_x(x_img):
    """x_img [H, W, C] f32 -> (x0, xu, xd, xcm) bf16 device images."""
    bf = ml_dtypes.bfloat16
    xb = x_img.astype(bf)

    def slot_img(src):
        img = np.zeros((128, SLOTS, C), dtype=bf)
        img[0:64, 1:34, :] = src[:, 0:33, :]      # wq0: slot s <-> w = s-1
        img[64:128, 0:33, :] = src[:, 31:64, :]   # wq1: slot s <-> w = 31+s
        return img.reshape(128, XF)

    zr = np.zeros((1, W, C), dtype=bf)
    x0 = slot_img(xb)
    xu = slot_img(np.concatenate([xb[1:], zr], axis=0))    # row h+1
    xd = slot_img(np.concatenate([zr, xb[:-1]], axis=0))   # row h-1
    # xcm[ch, (half*32+t)*128 + wq*64 + h] = x[h, 32wq+t, 128half+ch]
    a = xb.transpose(2, 1, 0)                  # [c, w, h]
    a = a.reshape(2, 128, 2, 32, H)            # [half, ch, wq, t, h]
    a = a.transpose(1, 0, 3, 2, 4)             # [ch, half, t, wq, h]
    xcm = np.ascontiguousarray(a.reshape(128, 8192))
    return x0, xu, xd, xcm


def kernel(x, W1, b1, W2, b2, trace=False):
    from concourse.bass_utils import run_bass_kernel_spmd

    nc = _get_program()
    bf = ml_dtypes.bfloat16
    w1_h = np.ascontiguousarray(W1.astype(bf).reshape(2, 128, D))
    b1_h = np.ascontiguousarray(b1.astype(bf).reshape(1, D))
    w2_h = np.ascontiguousarray(
        np.concatenate([W2, b2[None, :]], axis=0).astype(bf))
    in_maps = []
    for i in range(NCORES):
        x0, xu, xd, xcm = _host_x(np.asarray(x[i], dtype=np.float32))
        in_maps.append({
            "x0": x0, "xu": xu, "xd": xd, "xcm": xcm,
            "w1": w1_h, "b1row": b1_h, "w2aug": w2_h,
        })
    res = run_bass_kernel_spmd(nc, in_maps, core_ids=list(range(NCORES)),
                               trace=trace)
    out = np.stack([res.results[i]["out"] for i in range(NCORES)], axis=0)
    out = out.astype(np.float32)
    if trace:
        return out, res
    return out
